# revision 4
# baseline (speedup 1.0000x reference)
"""InterpretableMultiHeadAttention kernel for 8 Trainium2 NeuronCores.

The reference's cross-head mean before softmax collapses the per-head
attention into plain dense matmuls over the full model dim D:

    Qp = q @ Wq + bq ; Kp = k @ Wk + bk ; Vp = v @ Wv
    A  = (Qp @ Kp^T) / (H * sqrt(D/H))          # == mean over heads of scores
    W  = softmax(A, axis=-1)
    out = (W @ Vp) @ Wo + (bv @ Wo + bo)        # W rows sum to 1

Sharding: 8 cores = 4 batches x 2 query-row halves. Each core owns 1024
query rows of one batch and the full 2048 keys/values of that batch
(K/V projection duplicated across the core pair; no collectives).

Per-core device layouts (host pre-transposes, which is why the q/k/v
inputs arrive as qT/kT/vT with the contraction dim on partitions):
    QpT [D, SQ], KpT [D, SK]   <- projections, d_out on partitions
    Vp  [SK, D]                <- natural layout, sk on partitions
    A   [SQ, SK]               <- sq on partitions, softmax along free dim
    WT  [SK, SQ] bf16          <- DMA-transposed normalized weights
    ctxT[D, SQ], out [SQ, D]
"""

import os

import ml_dtypes
import numpy as np

import concourse.bass as bass
import concourse.tile as tile
from concourse import bacc, mybir
from concourse.bass_utils import run_bass_kernel_spmd

B, S, D, H = 4, 2048, 1024, 16
P = 128
SQ = S // 2          # query rows per core
SK = S               # key rows per core
ND = D // P          # 8 chunks of the model dim
NQ = SQ // P         # 8 query-row chunks
NK = SK // P         # 16 key-row chunks
FD = 512             # matmul moving free dim / PSUM bank
SCALE = 1.0 / (H * np.sqrt(D / H))

F32 = mybir.dt.float32
BF16 = mybir.dt.bfloat16

# matmul operand precision: "bf16", "f32r", or "f32"
MM_MODE = os.environ.get("IMHA_MM_MODE", "bf16")

_NC_CACHE = {}


def _mm_dt():
    return {"bf16": BF16, "f32r": mybir.dt.float32r, "f32": F32}[MM_MODE]


def _np_dt():
    return ml_dtypes.bfloat16 if MM_MODE == "bf16" else np.float32


def _build_program():
    mm_dt = _mm_dt()
    nc = bacc.Bacc("TRN2", target_bir_lowering=False, debug=False, num_devices=8)

    qT = nc.declare_dram_parameter("qT", [D, SQ], mm_dt, isOutput=False)[:]
    kT = nc.declare_dram_parameter("kT", [D, SK], mm_dt, isOutput=False)[:]
    vT = nc.declare_dram_parameter("vT", [D, SK], mm_dt, isOutput=False)[:]
    Wq = nc.declare_dram_parameter("Wq", [D, D], mm_dt, isOutput=False)[:]
    Wk = nc.declare_dram_parameter("Wk", [D, D], mm_dt, isOutput=False)[:]
    Wv = nc.declare_dram_parameter("Wv", [D, D], mm_dt, isOutput=False)[:]
    Wo = nc.declare_dram_parameter("Wo", [D, D], mm_dt, isOutput=False)[:]
    bqt = nc.declare_dram_parameter("bqt", [P, ND], F32, isOutput=False)[:]
    bkt = nc.declare_dram_parameter("bkt", [P, ND], F32, isOutput=False)[:]
    c0 = nc.declare_dram_parameter("c0", [P, D], F32, isOutput=False)[:]
    w_out = nc.declare_dram_parameter("w", [SQ, SK], F32, isOutput=True)[:]
    o_out = nc.declare_dram_parameter("out", [SQ, D], F32, isOutput=True)[:]

    Identity = mybir.ActivationFunctionType.Identity
    Copy = mybir.ActivationFunctionType.Copy
    Exp = mybir.ActivationFunctionType.Exp

    with tile.TileContext(nc) as tc:
        with (
            tc.tile_pool(name="persist", bufs=1) as persist,
            tc.tile_pool(name="psum", bufs=8, space="PSUM") as psum,
            tc.tile_pool(name="consts", bufs=1) as consts,
        ):
            QpT = persist.tile([P, ND, SQ], mm_dt)
            KpT = persist.tile([P, ND, SK], mm_dt)
            Vp = persist.tile([P, NK, D], mm_dt)
            WT = persist.tile([P, NK, SQ], BF16 if MM_MODE == "bf16" else mm_dt)
            ctxT = persist.tile([P, ND, SQ], mm_dt)

            bq_sb = consts.tile([P, ND], F32)
            nc.sync.dma_start(bq_sb[:], bqt)
            bk_sb = consts.tile([P, ND], F32)
            nc.sync.dma_start(bk_sb[:], bkt)
            c0_sb = consts.tile([P, D], F32)
            nc.sync.dma_start(c0_sb[:], c0)

            def proj_T(dst, w_dram, x_dram, n_free, bias_sb, scope):
                # dst[d_out, s] = (W^T @ xT)[d_out, s] + bias[d_out]
                with nc.named_scope(scope), tc.tile_pool(name=scope, bufs=1) as pp:
                    w_sb = pp.tile([P, ND, D], mm_dt, tag="w")
                    nc.sync.dma_start(w_sb[:], w_dram.rearrange("(o p) n -> p o n", p=P))
                    x_sb = pp.tile([P, ND, n_free], mm_dt, tag="x")
                    nc.sync.dma_start(x_sb[:], x_dram.rearrange("(o p) n -> p o n", p=P))
                    nf = n_free // FD
                    for j in range(ND):
                        for f in range(nf):
                            ps = psum.tile([P, FD], F32, tag="ps")
                            for i in range(ND):
                                nc.tensor.matmul(
                                    ps,
                                    lhsT=w_sb[:, i, j * P:(j + 1) * P],
                                    rhs=x_sb[:, i, f * FD:(f + 1) * FD],
                                    start=(i == 0),
                                    stop=(i == ND - 1),
                                )
                            if bias_sb is not None:
                                nc.scalar.activation(
                                    out=dst[:, j, f * FD:(f + 1) * FD],
                                    in_=ps,
                                    func=Identity,
                                    bias=bias_sb[:, j:j + 1],
                                    scale=1.0,
                                )
                            else:
                                nc.scalar.activation(
                                    out=dst[:, j, f * FD:(f + 1) * FD],
                                    in_=ps, func=Copy,
                                )

            proj_T(QpT, Wq, qT, SQ, bq_sb, "proj_q")
            proj_T(KpT, Wk, kT, SK, bk_sb, "proj_k")

            # Vp[sk, d_out] natural: lhsT = vT[d_in, sk] chunks, rhs = Wv
            with nc.named_scope("proj_v"), tc.tile_pool(name="pv", bufs=1) as pv:
                wv_sb = pv.tile([P, ND, D], mm_dt, tag="w")
                nc.sync.dma_start(wv_sb[:], Wv.rearrange("(o p) n -> p o n", p=P))
                vT_sb = pv.tile([P, ND, SK], mm_dt, tag="x")
                nc.sync.dma_start(vT_sb[:], vT.rearrange("(o p) n -> p o n", p=P))
                for kk in range(NK):
                    for f in range(D // FD):
                        ps = psum.tile([P, FD], F32, tag="ps")
                        for i in range(ND):
                            nc.tensor.matmul(
                                ps,
                                lhsT=vT_sb[:, i, kk * P:(kk + 1) * P],
                                rhs=wv_sb[:, i, f * FD:(f + 1) * FD],
                                start=(i == 0),
                                stop=(i == ND - 1),
                            )
                        nc.scalar.activation(
                            out=Vp[:, kk, f * FD:(f + 1) * FD], in_=ps, func=Copy,
                        )

            # scores + softmax + transposed weights, one 128-row query chunk
            # at a time
            with nc.named_scope("softmax"), tc.tile_pool(name="sm", bufs=2) as sm:
                for m in range(NQ):
                    pss = []
                    for t in range(SK // FD):
                        ps = psum.tile([P, FD], F32, tag="ps")
                        for i in range(ND):
                            nc.tensor.matmul(
                                ps,
                                lhsT=QpT[:, i, m * P:(m + 1) * P],
                                rhs=KpT[:, i, t * FD:(t + 1) * FD],
                                start=(i == 0),
                                stop=(i == ND - 1),
                            )
                        pss.append(ps)
                    expA = sm.tile([P, SK], F32, tag="expA")
                    sums4 = sm.tile([P, SK // FD], F32, tag="sums4")
                    for t in range(SK // FD):
                        nc.scalar.activation(
                            out=expA[:, t * FD:(t + 1) * FD],
                            in_=pss[t],
                            func=Exp,
                            scale=float(SCALE),
                            accum_out=sums4[:, t:t + 1],
                        )
                    recip = sm.tile([P, 1], F32, tag="recip")
                    nc.vector.reduce_sum(recip, sums4, axis=mybir.AxisListType.X)
                    nc.vector.reciprocal(recip, recip)
                    wn32 = sm.tile([P, SK], F32, tag="wn32")
                    nc.scalar.activation(
                        out=wn32, in_=expA, func=Copy, scale=recip,
                    )
                    nc.sync.dma_start(w_out[m * P:(m + 1) * P, :], wn32)
                    wnbf = sm.tile([P, SK], WT.dtype, tag="wnbf")
                    nc.vector.tensor_scalar_mul(wnbf, expA, recip)
                    for t in range(NK):
                        nc.sync.dma_start_transpose(
                            WT[:, t, m * P:(m + 1) * P],
                            wnbf[:, t * P:(t + 1) * P],
                        )

            # ctxT[d_out, sq] = Vp^T @ WT
            with nc.named_scope("ctx"):
                for j in range(ND):
                    for f in range(SQ // FD):
                        ps = psum.tile([P, FD], F32, tag="ps")
                        for kk in range(NK):
                            nc.tensor.matmul(
                                ps,
                                lhsT=Vp[:, kk, j * P:(j + 1) * P],
                                rhs=WT[:, kk, f * FD:(f + 1) * FD],
                                start=(kk == 0),
                                stop=(kk == NK - 1),
                            )
                        nc.scalar.activation(
                            out=ctxT[:, j, f * FD:(f + 1) * FD], in_=ps, func=Copy,
                        )

            # out[sq, d_out] = ctxT^T @ Wo + c0
            with nc.named_scope("out"), tc.tile_pool(name="po", bufs=2) as po:
                wo_sb = consts.tile([P, ND, D], mm_dt)
                nc.sync.dma_start(wo_sb[:], Wo.rearrange("(o p) n -> p o n", p=P))
                for m in range(NQ):
                    osb = po.tile([P, D], F32, tag="osb")
                    for f in range(D // FD):
                        ps = psum.tile([P, FD], F32, tag="ps")
                        for i in range(ND):
                            nc.tensor.matmul(
                                ps,
                                lhsT=ctxT[:, i, m * P:(m + 1) * P],
                                rhs=wo_sb[:, i, f * FD:(f + 1) * FD],
                                start=(i == 0),
                                stop=(i == ND - 1),
                            )
                        nc.vector.tensor_add(
                            out=osb[:, f * FD:(f + 1) * FD],
                            in0=ps,
                            in1=c0_sb[:, f * FD:(f + 1) * FD],
                        )
                    nc.sync.dma_start(o_out[m * P:(m + 1) * P, :], osb)

    nc.compile()
    return nc


def _get_program():
    if MM_MODE not in _NC_CACHE:
        _NC_CACHE[MM_MODE] = _build_program()
    return _NC_CACHE[MM_MODE]


def kernel(q, k, v, Wq, bq, Wk, bk, Wv, bv, Wo, bo, n_heads, **run_kwargs):
    assert int(n_heads) == H
    q = np.asarray(q, np.float32)
    k = np.asarray(k, np.float32)
    v = np.asarray(v, np.float32)
    Wq = np.asarray(Wq, np.float32)
    Wk = np.asarray(Wk, np.float32)
    Wv = np.asarray(Wv, np.float32)
    Wo = np.asarray(Wo, np.float32)
    bq = np.asarray(bq, np.float32)
    bk = np.asarray(bk, np.float32)
    bv = np.asarray(bv, np.float32)
    bo = np.asarray(bo, np.float32)

    cast_dt = _np_dt()
    c0row = (bv.astype(np.float64) @ Wo.astype(np.float64)
             + bo.astype(np.float64)).astype(np.float32)
    c0_full = np.ascontiguousarray(np.tile(c0row[None, :], (P, 1)))
    bqt = np.ascontiguousarray(bq.reshape(ND, P).T)
    bkt = np.ascontiguousarray(bk.reshape(ND, P).T)
    Wq_c = np.ascontiguousarray(Wq.astype(cast_dt))
    Wk_c = np.ascontiguousarray(Wk.astype(cast_dt))
    Wv_c = np.ascontiguousarray(Wv.astype(cast_dt))
    Wo_c = np.ascontiguousarray(Wo.astype(cast_dt))

    in_maps = []
    for c in range(8):
        b, hh = divmod(c, 2)
        in_maps.append({
            "qT": np.ascontiguousarray(q[b, hh * SQ:(hh + 1) * SQ, :].T.astype(cast_dt)),
            "kT": np.ascontiguousarray(k[b].T.astype(cast_dt)),
            "vT": np.ascontiguousarray(v[b].T.astype(cast_dt)),
            "Wq": Wq_c, "Wk": Wk_c, "Wv": Wv_c, "Wo": Wo_c,
            "bqt": bqt, "bkt": bkt, "c0": c0_full,
        })

    nc = _get_program()
    res = run_bass_kernel_spmd(nc, in_maps, core_ids=list(range(8)), **run_kwargs)

    out = np.empty((B, S, D), np.float32)
    w = np.empty((B, S, S), np.float32)
    for c in range(8):
        b, hh = divmod(c, 2)
        out[b, hh * SQ:(hh + 1) * SQ] = res.results[c]["out"]
        w[b, hh * SQ:(hh + 1) * SQ] = res.results[c]["w"]
    kernel.last_results = res
    return out, w
